# revision 24
# baseline (speedup 1.0000x reference)
# Block-diagonal (segmented) attention for Trainium2, head-parallel over 8 cores.
#
# Math: out[l, e] = softmax_m(q[l] @ k[m]^T * scale + bias[l, m]) @ v[m]
# with bias = 0 within a segment, -10000 across segments. exp(-10000 + s)
# underflows to exactly 0.0 in fp32, so only the diagonal blocks contribute;
# we compute exactly those (1/8 of the dense work for the 8x512 case).
#
# Sharding: one head per NeuronCore (H=8 across 8 cores), no collectives.
#
# Per-core layout (one head per core):
#   qT, kT  : [64, L] host-transposed, cast to the matmul dtype (fp16 default)
#   v1      : v with a ones column appended; aligned path pre-swizzles to
#             [128, L/128, E+1] so one DMA loads every k-tile; general path
#             keeps [L, E+1] with per-tile row loads
#   outT    : [E, L] fp32 (host transposes back)
#
# Per segment [s, e), per q-tile of <=512 columns (all transpose-free):
#   S^T tile  = matmul(lhsT=kT[:, ktile], rhs=qT[:, qtile])    -> PSUM [kn, qn]
#   P~        = exp(S^T * scale - 4)        (ScalarE, PSUM -> SBUF, fp16;
#               the constant shift cancels in softmax and keeps fp16 range)
#   accum     = matmul(lhsT=v1[ktile], rhs=P~) accumulated     -> PSUM [65, qn]
#               (row 64 = ones column = softmax denominators)
#   normalize : outT = accum[0:64] * (1 / accum[64]); with norm_mode=deferred*
#               the per-segment work is just a DVE reciprocal + copy, and the
#               partition-broadcast + multiply + store run in a few batches
#               (deferredg: geometric batches so the serial tail flush is one
#               segment wide).
#
# Softmax needs no per-row max subtraction: scores*scale ~ N(0,1), so exp()
# stays in a tiny dynamic range (measured max 6.0 for the reference inputs).
#
# Measured on the axon-tunneled TRN2 cores: fp16 matmuls are ~3x faster than
# float32r end to end (f32r weight loads are slow); max abs err ~5e-4 vs the
# fp32 reference (scale ~0.8). Loop-slope differencing with a barebones
# baseline puts the body at ~18us/execution (~= the ScalarE exp roofline:
# 2.1M exp elements/core at 1 elem/lane/cycle); raw sustained-loop slope
# reads ~55-60us because the For_i barrier machinery alone costs ~44us/iter.

import numpy as np

L = 4096
H = 8
E = 64
P = 128
NCORES = 8
SCALE = 0.125  # 1/sqrt(E)
QTILE = 512

# tunables (model-swept)
CFG = dict(
    row_tiled=False,    # pack the two 64-contraction S-matmuls via tile_position
    load_chunks=0,      # 0 = graded chunks (512,512,1024,2048); N = equal
    store_engine="sync",  # "sync" | "scalar" | "gpsimd"
    psum_s_bufs=3,
    psum_o_bufs=2,
    p_bufs=4,
    misc_bufs=4,
    norm_mode="deferredg",  # "per_seg" | "deferredN" | "deferredg"
    mm_dtype="fp16",      # "f32r" | "bf16" | "fp16" (16-bit halves DMA; fp16
                          # keeps 10 mantissa bits -> ~1e-3 err vs 4e-3 bf16)
    # ablation flags (timing experiments only; break numerics)
    skip_loads=False,
    skip_smm=False,
    skip_exp=False,
    skip_pv=False,
    skip_norm=False,
    skip_store=False,
)

_prog_cache = {}


def _segment_bounds(seg_ids):
    s = np.asarray(seg_ids).reshape(-1)
    assert s.shape[0] == L
    d = np.diff(s)
    assert np.all(d >= 0), "seg_ids must be sorted"
    change = (np.flatnonzero(d) + 1).tolist()
    starts = [0] + change
    ends = change + [L]
    return tuple(zip(starts, ends))


def _aligned(bounds):
    return all(s % P == 0 for (s, e) in bounds)


def _build(bounds, reps=1, cfg=None, loop_reps=0):
    """Build + compile the per-core Bass program for the given segment bounds.

    reps > 1 statically unrolls the whole body (for wall-clock timing).
    loop_reps > 0 wraps the body in a dynamic For_i loop instead (constant
    NEFF size, for clean wall-clock differencing)."""
    from contextlib import ExitStack

    import concourse.bacc as bacc
    import concourse.tile as tile
    from concourse import mybir

    cfg = dict(CFG, **(cfg or {}))
    f32 = mybir.dt.float32
    f32r = mybir.dt.float32r
    Exp = mybir.ActivationFunctionType.Exp

    aligned = _aligned(bounds)
    # fp32r matmuls have ISA shape restrictions; only use them on the fully
    # 512-aligned fast path (all tiles full-size). Fallback: plain fp32.
    fast = all(s % QTILE == 0 for (s, e) in bounds)
    # row-tiled packing needs all k-tiles full (128) and duplicated q/k rows
    row_tiled = cfg["row_tiled"] and aligned
    QK_P = 2 * E if row_tiled else E
    if cfg["mm_dtype"] == "bf16":
        mmdt = mybir.dt.bfloat16
    elif cfg["mm_dtype"] == "fp16":
        mmdt = mybir.dt.float16
    else:
        mmdt = f32r if fast else f32
    # constant shift inside exp (softmax is shift-invariant): keeps P~ well
    # inside fp16 range (overflow would need score*scale >= 11 + shift)
    exp_bias = -4.0 if cfg["mm_dtype"] == "fp16" else 0.0

    nc = bacc.Bacc(
        "TRN2", target_bir_lowering=False, debug=False, num_devices=NCORES
    )
    qT = nc.dram_tensor("qT", [QK_P, L], mmdt, kind="ExternalInput").ap()
    kT = nc.dram_tensor("kT", [QK_P, L], mmdt, kind="ExternalInput").ap()
    if aligned:
        v1 = nc.dram_tensor("v1", [P, L // P, E + 1], mmdt, kind="ExternalInput").ap()
    else:
        v1 = nc.dram_tensor("v1", [L, E + 1], mmdt, kind="ExternalInput").ap()
    outT = nc.dram_tensor("outT", [E, L], f32, kind="ExternalOutput").ap()

    max_seg = max(e - s for (s, e) in bounds)
    max_nk = (max_seg + P - 1) // P

    store_eng = {"sync": "sync", "scalar": "scalar", "gpsimd": "gpsimd"}[
        cfg["store_engine"]
    ]

    with ExitStack() as ctx:
        tc = ctx.enter_context(tile.TileContext(nc))
        singles = ctx.enter_context(tc.tile_pool(name="singles", bufs=1))
        vpool = ctx.enter_context(tc.tile_pool(name="vpool", bufs=2))
        ppool = ctx.enter_context(tc.tile_pool(name="ppool", bufs=cfg["p_bufs"]))
        opool = ctx.enter_context(tc.tile_pool(name="opool", bufs=cfg["misc_bufs"]))
        rpool = ctx.enter_context(tc.tile_pool(name="rpool", bufs=cfg["misc_bufs"]))
        normpool = ctx.enter_context(tc.tile_pool(name="normpool", bufs=2))
        psum_s = ctx.enter_context(
            tc.tile_pool(name="psum_s", bufs=cfg["psum_s_bufs"], space="PSUM")
        )
        psum_o = ctx.enter_context(
            tc.tile_pool(name="psum_o", bufs=cfg["psum_o_bufs"], space="PSUM")
        )

        exp_bias_sb = None
        if exp_bias != 0.0:
            exp_bias_sb = singles.tile([P, 1], f32, tag="exp_bias")
            nc.vector.memset(exp_bias_sb, exp_bias)

        def ebias(kn):
            if exp_bias_sb is None:
                return 0.0
            return exp_bias_sb[0:kn]

        def touch(ap):
            # tiny write so ablated builds still allocate the tile
            nc.vector.memset(ap, 0.0)

        def emit_norm_flush(o_all, r_all, lo, hi):
            # one broadcast + one multiply + one store for columns [lo, hi)
            w = hi - lo
            rb = normpool.tile([E, L], f32, tag="rb_all")
            nc.gpsimd.partition_broadcast(
                rb[:, lo:hi], r_all[0:1, lo:hi]
            )
            nc.vector.tensor_mul(
                o_all[:, lo:hi], o_all[:, lo:hi], rb[:, lo:hi]
            )
            getattr(nc, store_eng).dma_start(
                out=outT[:, lo:hi], in_=o_all[:, lo:hi]
            )

        def body():
            # chunked whole-tensor input loads (SP HWDGE ring)
            qT_sb = singles.tile([QK_P, L], mmdt, tag="qT")
            kT_sb = singles.tile([QK_P, L], mmdt, tag="kT")
            nchunk = cfg["load_chunks"]
            if nchunk == 0:
                # graded: small first chunks so compute starts early
                edges = [0, 512, 1024, 2048, L]
            else:
                cw = L // nchunk
                edges = [c * cw for c in range(nchunk)] + [L]
            if not cfg["skip_loads"]:
                for c in range(len(edges) - 1):
                    sl = slice(edges[c], edges[c + 1])
                    nc.sync.dma_start(out=qT_sb[:, sl], in_=qT[:, sl])
                    nc.sync.dma_start(out=kT_sb[:, sl], in_=kT[:, sl])
            if aligned:
                v_all = singles.tile([P, L // P, E + 1], mmdt, tag="v")
                if not cfg["skip_loads"]:
                    nc.sync.dma_start(out=v_all, in_=v1)
            norm_mode = cfg["norm_mode"]
            if norm_mode != "per_seg":
                o_all = normpool.tile([E, L], f32, tag="o_all")
                r_all = normpool.tile([1, L], f32, tag="r_all")
                nseg = len(bounds)
                if norm_mode == "deferredg":
                    # geometric: halve the remaining segments each flush so
                    # the final (serial-tail) flush is a single segment
                    idxs = []
                    lo = 0
                    while lo < nseg:
                        step = max(1, (nseg - lo) // 2)
                        if nseg - lo <= 2:
                            step = 1
                        lo += step
                        idxs.append(lo - 1)
                    flush_pts = [bounds[i][1] for i in idxs]
                else:
                    nbatch = int(norm_mode[len("deferred"):] or "1")
                    flush_pts = [
                        bounds[nseg * (b + 1) // nbatch - 1][1]
                        for b in range(nbatch)
                    ]
                flushed = 0
            if cfg["skip_loads"]:
                # tiny loads keep tiles verifier-legal (f32r needs a rounding
                # producer) while eliminating ~all DMA traffic
                nc.sync.dma_start(out=qT_sb[:, 0:8], in_=qT[:, 0:8])
                nc.sync.dma_start(out=kT_sb[:, 0:8], in_=kT[:, 0:8])
                if aligned:
                    nc.sync.dma_start(out=v_all[:, 0, 0:8], in_=v1[:, 0, 0:8])

            for (s, e) in bounds:
                seg = e - s
                if seg <= 0:
                    continue
                nk = (seg + P - 1) // P

                if aligned:
                    def v_tile(i, kn):
                        return v_all[:, (s // P) + i, :]
                else:
                    v_s = vpool.tile([P, max_nk, E + 1], mmdt, tag="vseg")
                    for i in range(nk):
                        k0 = s + i * P
                        kn = min(P, e - k0)
                        nc.sync.dma_start(
                            out=v_s[0:kn, i, :], in_=v1[k0 : k0 + kn, :]
                        )

                    def v_tile(i, kn):
                        return v_s[0:kn, i, :]

                for q0 in range(s, e, QTILE):
                    qn = min(QTILE, e - q0)

                    po = psum_o.tile([E + 1, QTILE], f32, tag="po")

                    # S^T = K Q^T, then P~ = exp(S^T * scale)
                    npair = (nk + 1) // 2
                    p_tiles = []
                    for j in range(npair):
                        ps = psum_s.tile([P, 2 * QTILE], f32, tag="ps")
                        p_sb = ppool.tile([P, 2 * QTILE], mmdt, tag="p")
                        slots = []
                        for t in range(2):
                            i = 2 * j + t
                            if i >= nk:
                                continue
                            k0 = s + i * P
                            kn = min(P, e - k0)
                            if cfg["skip_smm"]:
                                if t == 0:
                                    touch(ps[:, 0:8])
                                slots.append((t, kn))
                                continue
                            if row_tiled:
                                # two concurrent 64-row matmuls in the PE
                                # array: tile A rows 0-63, tile B rows 64-127
                                rowoff = t * E
                                nc.tensor.matmul(
                                    ps[0:kn, t * QTILE : t * QTILE + qn],
                                    lhsT=kT_sb[
                                        rowoff : rowoff + E, k0 : k0 + kn
                                    ],
                                    rhs=qT_sb[
                                        rowoff : rowoff + E, q0 : q0 + qn
                                    ],
                                    start=True,
                                    stop=True,
                                    tile_position=(rowoff, 0),
                                )
                            else:
                                nc.tensor.matmul(
                                    ps[0:kn, t * QTILE : t * QTILE + qn],
                                    lhsT=kT_sb[0:E, k0 : k0 + kn],
                                    rhs=qT_sb[0:E, q0 : q0 + qn],
                                    start=True,
                                    stop=True,
                                )
                            slots.append((t, kn))
                        if cfg["skip_exp"]:
                            nc.scalar.activation(
                                out=p_sb[:, 0:8], in_=ps[:, 0:8],
                                func=Exp, scale=SCALE,
                            )
                        elif (
                            len(slots) == 2
                            and all(kn == P for (_, kn) in slots)
                            and qn == QTILE
                        ):
                            nc.scalar.activation(
                                out=p_sb, in_=ps, func=Exp, scale=SCALE,
                                bias=ebias(P),
                            )
                        else:
                            for (t, kn) in slots:
                                nc.scalar.activation(
                                    out=p_sb[0:kn, t * QTILE : t * QTILE + qn],
                                    in_=ps[0:kn, t * QTILE : t * QTILE + qn],
                                    func=Exp,
                                    scale=SCALE,
                                    bias=ebias(kn),
                                )
                        p_tiles.append(p_sb)

                    # out^T (+ denominators) = [V | 1]^T @ P~, accumulated
                    if cfg["skip_pv"]:
                        touch(po[:, 0:8])
                    for i in range(nk):
                        if cfg["skip_pv"]:
                            break
                        k0 = s + i * P
                        kn = min(P, e - k0)
                        p_sb = p_tiles[i // 2]
                        off = (i % 2) * QTILE
                        nc.tensor.matmul(
                            po[0 : E + 1, 0:qn],
                            lhsT=v_tile(i, kn),
                            rhs=p_sb[0:kn, off : off + qn],
                            start=(i == 0),
                            stop=(i == nk - 1),
                        )

                    # normalize: outT = po[0:64] * (1 / po[64])
                    if norm_mode != "per_seg":
                        nc.vector.reciprocal(
                            r_all[0:1, q0 : q0 + qn], po[E : E + 1, 0:qn]
                        )
                        nc.vector.tensor_copy(
                            o_all[:, q0 : q0 + qn], po[0:E, 0:qn]
                        )
                        continue
                    o_sb = opool.tile([E, QTILE], f32, tag="o")
                    if cfg["skip_norm"] and not cfg["skip_store"]:
                        touch(o_sb[:, 0:8])
                    if not cfg["skip_norm"]:
                        r_sb = rpool.tile([1, QTILE], f32, tag="r")
                        nc.vector.reciprocal(r_sb[:, 0:qn], po[E : E + 1, 0:qn])
                        rb_sb = rpool.tile([E, QTILE], f32, tag="rb")
                        nc.gpsimd.partition_broadcast(
                            rb_sb[:, 0:qn], r_sb[0:1, 0:qn]
                        )
                        nc.vector.tensor_mul(
                            o_sb[:, 0:qn], po[0:E, 0:qn], rb_sb[:, 0:qn]
                        )
                    if not cfg["skip_store"]:
                        getattr(nc, store_eng).dma_start(
                            out=outT[:, q0 : q0 + qn], in_=o_sb[:, 0:qn]
                        )

            if norm_mode != "per_seg":
                for pt in flush_pts:
                    emit_norm_flush(o_all, r_all, flushed, pt)
                    flushed = pt

        if loop_reps > 0:
            with tc.For_i(0, loop_reps, 1):
                body()
        else:
            for _ in range(reps):
                body()

    nc.compile()
    return nc


def _get_program(bounds, reps=1):
    key = (bounds, reps)
    if key not in _prog_cache:
        _prog_cache[key] = _build(bounds, reps=reps)
    return _prog_cache[key]


def _make_in_maps(q, k, v, bounds):
    aligned = _aligned(bounds)
    row_tiled = CFG["row_tiled"] and aligned
    if CFG["mm_dtype"] == "bf16":
        import ml_dtypes

        dt = ml_dtypes.bfloat16
    elif CFG["mm_dtype"] == "fp16":
        dt = np.float16
    else:
        dt = np.float32
    in_maps = []
    for h in range(H):
        qh = np.ascontiguousarray(q[0, :, h, :].T.astype(dt))  # [E, L]
        kh = np.ascontiguousarray(k[0, :, h, :].T.astype(dt))  # [E, L]
        if row_tiled:
            qh = np.ascontiguousarray(np.concatenate([qh, qh], axis=0))
            kh = np.ascontiguousarray(np.concatenate([kh, kh], axis=0))
        v1h = np.empty((L, E + 1), dtype=dt)
        v1h[:, :E] = v[0, :, h, :].astype(dt)
        v1h[:, E] = 1.0
        if aligned:
            # swizzle so one SBUF partition holds one row of every k-tile:
            # v1_sw[p, g, e] = v1[g*128 + p, e]
            v1h = np.ascontiguousarray(
                v1h.reshape(L // P, P, E + 1).transpose(1, 0, 2)
            )
        in_maps.append({"qT": qh, "kT": kh, "v1": v1h})
    return in_maps


def kernel(q, k, v, seg_ids):
    from concourse import bass_utils

    q = np.asarray(q, dtype=np.float32)
    k = np.asarray(k, dtype=np.float32)
    v = np.asarray(v, dtype=np.float32)
    seg_ids = np.asarray(seg_ids)

    bounds = _segment_bounds(seg_ids)
    nc = _get_program(bounds)
    in_maps = _make_in_maps(q, k, v, bounds)

    res = bass_utils.run_bass_kernel_spmd(nc, in_maps, core_ids=list(range(NCORES)))

    out = np.empty((1, L, H, E), dtype=np.float32)
    for h in range(H):
        out[0, :, h, :] = res.results[h]["outT"].T
    return out


# revision 26
# speedup vs baseline: 1.0718x; 1.0718x over previous
# Block-diagonal (segmented) attention for Trainium2, head-parallel over 8 cores.
#
# Math: out[l, e] = softmax_m(q[l] @ k[m]^T * scale + bias[l, m]) @ v[m]
# with bias = 0 within a segment, -10000 across segments. exp(-10000 + s)
# underflows to exactly 0.0 in fp32, so only the diagonal blocks contribute;
# we compute exactly those (1/8 of the dense work for the 8x512 case).
#
# Sharding: one head per NeuronCore (H=8 across 8 cores), no collectives.
#
# Per-core layout (one head per core):
#   qT, kT  : [64, L] host-transposed, cast to the matmul dtype (fp16 default)
#   v1      : v with a ones column appended; aligned path pre-swizzles to
#             [128, L/128, E+1] so one DMA loads every k-tile; general path
#             keeps [L, E+1] with per-tile row loads
#   outT    : [E, L] fp32 (host transposes back)
#
# Per segment [s, e), per q-tile of <=512 columns (all transpose-free):
#   S^T tile  = matmul(lhsT=kT[:, ktile], rhs=qT[:, qtile])    -> PSUM [kn, qn]
#   P~        = exp(S^T * scale - 4)        (ScalarE, PSUM -> SBUF, fp16;
#               the constant shift cancels in softmax and keeps fp16 range)
#   accum     = matmul(lhsT=v1[ktile], rhs=P~) accumulated     -> PSUM [65, qn]
#               (row 64 = ones column = softmax denominators)
#   normalize : outT = accum[0:64] * (1 / accum[64]); with norm_mode=deferred*
#               the per-segment work is just a DVE reciprocal + copy, and the
#               partition-broadcast + multiply + store run in a few batches
#               (deferredg: geometric batches so the serial tail flush is one
#               segment wide).
#
# Softmax needs no per-row max subtraction: scores*scale ~ N(0,1), so exp()
# stays in a tiny dynamic range (measured max 6.0 for the reference inputs).
#
# Measured on the axon-tunneled TRN2 cores: fp16 matmuls are ~3x faster than
# float32r end to end (f32r weight loads are slow); max abs err ~5e-4 vs the
# fp32 reference (scale ~0.8). Loop-slope differencing with a barebones
# baseline puts the body at ~18us/execution (~= the ScalarE exp roofline:
# 2.1M exp elements/core at 1 elem/lane/cycle); raw sustained-loop slope
# reads ~55-60us because the For_i barrier machinery alone costs ~44us/iter.

import numpy as np

L = 4096
H = 8
E = 64
P = 128
NCORES = 8
SCALE = 0.125  # 1/sqrt(E)
QTILE = 512

# tunables (model-swept)
CFG = dict(
    row_tiled=False,    # pack the two 64-contraction S-matmuls via tile_position
    load_chunks=0,      # 0 = graded chunks (512,512,1024,2048); N = equal
    store_engine="sync",  # "sync" | "scalar" | "gpsimd"
    psum_s_bufs=3,
    psum_o_bufs=2,
    p_bufs=4,
    misc_bufs=4,
    norm_mode="deferredg",  # "per_seg" | "deferredN" | "deferredg"
    warmup_pe=0,        # dummy matmuls at t=0 to warm the PE HAM clock-gate.
                        # Measured NET-NEGATIVE (+6us): cold warmup matmuls
                        # run at 1.2GHz and outlast the load prologue, so the
                        # delay exceeds the ~1.7us ramp saving. Keep 0.
    mm_dtype="fp16",      # "f32r" | "bf16" | "fp16" (16-bit halves DMA; fp16
                          # keeps 10 mantissa bits -> ~1e-3 err vs 4e-3 bf16)
    # ablation flags (timing experiments only; break numerics)
    skip_loads=False,
    skip_smm=False,
    skip_exp=False,
    skip_pv=False,
    skip_norm=False,
    skip_store=False,
)

_prog_cache = {}


def _segment_bounds(seg_ids):
    s = np.asarray(seg_ids).reshape(-1)
    assert s.shape[0] == L
    d = np.diff(s)
    assert np.all(d >= 0), "seg_ids must be sorted"
    change = (np.flatnonzero(d) + 1).tolist()
    starts = [0] + change
    ends = change + [L]
    return tuple(zip(starts, ends))


def _aligned(bounds):
    return all(s % P == 0 for (s, e) in bounds)


def _build(bounds, reps=1, cfg=None, loop_reps=0):
    """Build + compile the per-core Bass program for the given segment bounds.

    reps > 1 statically unrolls the whole body (for wall-clock timing).
    loop_reps > 0 wraps the body in a dynamic For_i loop instead (constant
    NEFF size, for clean wall-clock differencing)."""
    from contextlib import ExitStack

    import concourse.bacc as bacc
    import concourse.tile as tile
    from concourse import mybir

    cfg = dict(CFG, **(cfg or {}))
    f32 = mybir.dt.float32
    f32r = mybir.dt.float32r
    Exp = mybir.ActivationFunctionType.Exp

    aligned = _aligned(bounds)
    # fp32r matmuls have ISA shape restrictions; only use them on the fully
    # 512-aligned fast path (all tiles full-size). Fallback: plain fp32.
    fast = all(s % QTILE == 0 for (s, e) in bounds)
    # row-tiled packing needs all k-tiles full (128) and duplicated q/k rows
    row_tiled = cfg["row_tiled"] and aligned
    QK_P = 2 * E if row_tiled else E
    if cfg["mm_dtype"] == "bf16":
        mmdt = mybir.dt.bfloat16
    elif cfg["mm_dtype"] == "fp16":
        mmdt = mybir.dt.float16
    else:
        mmdt = f32r if fast else f32
    # constant shift inside exp (softmax is shift-invariant): keeps P~ well
    # inside fp16 range (overflow would need score*scale >= 11 + shift)
    exp_bias = -4.0 if cfg["mm_dtype"] == "fp16" else 0.0

    nc = bacc.Bacc(
        "TRN2", target_bir_lowering=False, debug=False, num_devices=NCORES
    )
    qT = nc.dram_tensor("qT", [QK_P, L], mmdt, kind="ExternalInput").ap()
    kT = nc.dram_tensor("kT", [QK_P, L], mmdt, kind="ExternalInput").ap()
    if aligned:
        v1 = nc.dram_tensor("v1", [P, L // P, E + 1], mmdt, kind="ExternalInput").ap()
    else:
        v1 = nc.dram_tensor("v1", [L, E + 1], mmdt, kind="ExternalInput").ap()
    outT = nc.dram_tensor("outT", [E, L], f32, kind="ExternalOutput").ap()

    max_seg = max(e - s for (s, e) in bounds)
    max_nk = (max_seg + P - 1) // P

    store_eng = {"sync": "sync", "scalar": "scalar", "gpsimd": "gpsimd"}[
        cfg["store_engine"]
    ]

    with ExitStack() as ctx:
        tc = ctx.enter_context(tile.TileContext(nc))
        singles = ctx.enter_context(tc.tile_pool(name="singles", bufs=1))
        vpool = ctx.enter_context(tc.tile_pool(name="vpool", bufs=2))
        ppool = ctx.enter_context(tc.tile_pool(name="ppool", bufs=cfg["p_bufs"]))
        opool = ctx.enter_context(tc.tile_pool(name="opool", bufs=cfg["misc_bufs"]))
        rpool = ctx.enter_context(tc.tile_pool(name="rpool", bufs=cfg["misc_bufs"]))
        normpool = ctx.enter_context(tc.tile_pool(name="normpool", bufs=2))
        psum_s = ctx.enter_context(
            tc.tile_pool(name="psum_s", bufs=cfg["psum_s_bufs"], space="PSUM")
        )
        psum_o = ctx.enter_context(
            tc.tile_pool(name="psum_o", bufs=cfg["psum_o_bufs"], space="PSUM")
        )

        exp_bias_sb = None
        if exp_bias != 0.0:
            exp_bias_sb = singles.tile([P, 1], f32, tag="exp_bias")
            nc.vector.memset(exp_bias_sb, exp_bias)

        def ebias(kn):
            if exp_bias_sb is None:
                return 0.0
            return exp_bias_sb[0:kn]

        def touch(ap):
            # tiny write so ablated builds still allocate the tile
            nc.vector.memset(ap, 0.0)

        def emit_norm_flush(o_all, r_all, lo, hi):
            # one broadcast + one multiply + one store for columns [lo, hi)
            w = hi - lo
            rb = normpool.tile([E, L], f32, tag="rb_all")
            nc.gpsimd.partition_broadcast(
                rb[:, lo:hi], r_all[0:1, lo:hi]
            )
            nc.vector.tensor_mul(
                o_all[:, lo:hi], o_all[:, lo:hi], rb[:, lo:hi]
            )
            getattr(nc, store_eng).dma_start(
                out=outT[:, lo:hi], in_=o_all[:, lo:hi]
            )

        def body():
            # PE warmup: dependency-free matmuls on garbage SBUF so the HAM
            # clock-gate reaches 8/8 while the input DMAs are still landing.
            # The target psum_s slot is recycled by the real pipeline.
            nwarm = cfg["warmup_pe"]
            if nwarm > 0:
                warm_src = singles.tile([E, QTILE], mmdt, tag="warm")
                nc.vector.memset(warm_src, 0.0)
                warm_ps = psum_s.tile([P, 2 * QTILE], f32, tag="ps")
                for w in range(nwarm):
                    nc.tensor.matmul(
                        warm_ps[0:P, (w % 2) * QTILE : (w % 2) * QTILE + QTILE],
                        lhsT=warm_src[:, 0:P],
                        rhs=warm_src[:, 0:QTILE],
                        start=True,
                        stop=True,
                    )

            # chunked whole-tensor input loads (SP HWDGE ring)
            qT_sb = singles.tile([QK_P, L], mmdt, tag="qT")
            kT_sb = singles.tile([QK_P, L], mmdt, tag="kT")
            nchunk = cfg["load_chunks"]
            if nchunk == 0:
                # graded: small first chunks so compute starts early
                edges = [0, 512, 1024, 2048, L]
            else:
                cw = L // nchunk
                edges = [c * cw for c in range(nchunk)] + [L]
            if not cfg["skip_loads"]:
                for c in range(len(edges) - 1):
                    sl = slice(edges[c], edges[c + 1])
                    nc.sync.dma_start(out=qT_sb[:, sl], in_=qT[:, sl])
                    nc.sync.dma_start(out=kT_sb[:, sl], in_=kT[:, sl])
            if aligned:
                v_all = singles.tile([P, L // P, E + 1], mmdt, tag="v")
                if not cfg["skip_loads"]:
                    nc.sync.dma_start(out=v_all, in_=v1)
            norm_mode = cfg["norm_mode"]
            if norm_mode != "per_seg":
                o_all = normpool.tile([E, L], f32, tag="o_all")
                r_all = normpool.tile([1, L], f32, tag="r_all")
                nseg = len(bounds)
                if norm_mode == "deferredg":
                    # geometric: halve the remaining segments each flush so
                    # the final (serial-tail) flush is a single segment
                    idxs = []
                    lo = 0
                    while lo < nseg:
                        step = max(1, (nseg - lo) // 2)
                        if nseg - lo <= 2:
                            step = 1
                        lo += step
                        idxs.append(lo - 1)
                    flush_pts = [bounds[i][1] for i in idxs]
                else:
                    nbatch = int(norm_mode[len("deferred"):] or "1")
                    flush_pts = [
                        bounds[nseg * (b + 1) // nbatch - 1][1]
                        for b in range(nbatch)
                    ]
                flushed = 0
            if cfg["skip_loads"]:
                # tiny loads keep tiles verifier-legal (f32r needs a rounding
                # producer) while eliminating ~all DMA traffic
                nc.sync.dma_start(out=qT_sb[:, 0:8], in_=qT[:, 0:8])
                nc.sync.dma_start(out=kT_sb[:, 0:8], in_=kT[:, 0:8])
                if aligned:
                    nc.sync.dma_start(out=v_all[:, 0, 0:8], in_=v1[:, 0, 0:8])

            for (s, e) in bounds:
                seg = e - s
                if seg <= 0:
                    continue
                nk = (seg + P - 1) // P

                if aligned:
                    def v_tile(i, kn):
                        return v_all[:, (s // P) + i, :]
                else:
                    v_s = vpool.tile([P, max_nk, E + 1], mmdt, tag="vseg")
                    for i in range(nk):
                        k0 = s + i * P
                        kn = min(P, e - k0)
                        nc.sync.dma_start(
                            out=v_s[0:kn, i, :], in_=v1[k0 : k0 + kn, :]
                        )

                    def v_tile(i, kn):
                        return v_s[0:kn, i, :]

                for q0 in range(s, e, QTILE):
                    qn = min(QTILE, e - q0)

                    po = psum_o.tile([E + 1, QTILE], f32, tag="po")

                    # S^T = K Q^T, then P~ = exp(S^T * scale)
                    npair = (nk + 1) // 2
                    p_tiles = []
                    for j in range(npair):
                        ps = psum_s.tile([P, 2 * QTILE], f32, tag="ps")
                        p_sb = ppool.tile([P, 2 * QTILE], mmdt, tag="p")
                        slots = []
                        for t in range(2):
                            i = 2 * j + t
                            if i >= nk:
                                continue
                            k0 = s + i * P
                            kn = min(P, e - k0)
                            if cfg["skip_smm"]:
                                if t == 0:
                                    touch(ps[:, 0:8])
                                slots.append((t, kn))
                                continue
                            if row_tiled:
                                # two concurrent 64-row matmuls in the PE
                                # array: tile A rows 0-63, tile B rows 64-127
                                rowoff = t * E
                                nc.tensor.matmul(
                                    ps[0:kn, t * QTILE : t * QTILE + qn],
                                    lhsT=kT_sb[
                                        rowoff : rowoff + E, k0 : k0 + kn
                                    ],
                                    rhs=qT_sb[
                                        rowoff : rowoff + E, q0 : q0 + qn
                                    ],
                                    start=True,
                                    stop=True,
                                    tile_position=(rowoff, 0),
                                )
                            else:
                                nc.tensor.matmul(
                                    ps[0:kn, t * QTILE : t * QTILE + qn],
                                    lhsT=kT_sb[0:E, k0 : k0 + kn],
                                    rhs=qT_sb[0:E, q0 : q0 + qn],
                                    start=True,
                                    stop=True,
                                )
                            slots.append((t, kn))
                        if cfg["skip_exp"]:
                            nc.scalar.activation(
                                out=p_sb[:, 0:8], in_=ps[:, 0:8],
                                func=Exp, scale=SCALE,
                            )
                        elif (
                            len(slots) == 2
                            and all(kn == P for (_, kn) in slots)
                            and qn == QTILE
                        ):
                            nc.scalar.activation(
                                out=p_sb, in_=ps, func=Exp, scale=SCALE,
                                bias=ebias(P),
                            )
                        else:
                            for (t, kn) in slots:
                                nc.scalar.activation(
                                    out=p_sb[0:kn, t * QTILE : t * QTILE + qn],
                                    in_=ps[0:kn, t * QTILE : t * QTILE + qn],
                                    func=Exp,
                                    scale=SCALE,
                                    bias=ebias(kn),
                                )
                        p_tiles.append(p_sb)

                    # out^T (+ denominators) = [V | 1]^T @ P~, accumulated
                    if cfg["skip_pv"]:
                        touch(po[:, 0:8])
                    for i in range(nk):
                        if cfg["skip_pv"]:
                            break
                        k0 = s + i * P
                        kn = min(P, e - k0)
                        p_sb = p_tiles[i // 2]
                        off = (i % 2) * QTILE
                        nc.tensor.matmul(
                            po[0 : E + 1, 0:qn],
                            lhsT=v_tile(i, kn),
                            rhs=p_sb[0:kn, off : off + qn],
                            start=(i == 0),
                            stop=(i == nk - 1),
                        )

                    # normalize: outT = po[0:64] * (1 / po[64])
                    if norm_mode != "per_seg":
                        nc.vector.reciprocal(
                            r_all[0:1, q0 : q0 + qn], po[E : E + 1, 0:qn]
                        )
                        nc.vector.tensor_copy(
                            o_all[:, q0 : q0 + qn], po[0:E, 0:qn]
                        )
                        continue
                    o_sb = opool.tile([E, QTILE], f32, tag="o")
                    if cfg["skip_norm"] and not cfg["skip_store"]:
                        touch(o_sb[:, 0:8])
                    if not cfg["skip_norm"]:
                        r_sb = rpool.tile([1, QTILE], f32, tag="r")
                        nc.vector.reciprocal(r_sb[:, 0:qn], po[E : E + 1, 0:qn])
                        rb_sb = rpool.tile([E, QTILE], f32, tag="rb")
                        nc.gpsimd.partition_broadcast(
                            rb_sb[:, 0:qn], r_sb[0:1, 0:qn]
                        )
                        nc.vector.tensor_mul(
                            o_sb[:, 0:qn], po[0:E, 0:qn], rb_sb[:, 0:qn]
                        )
                    if not cfg["skip_store"]:
                        getattr(nc, store_eng).dma_start(
                            out=outT[:, q0 : q0 + qn], in_=o_sb[:, 0:qn]
                        )

            if norm_mode != "per_seg":
                for pt in flush_pts:
                    emit_norm_flush(o_all, r_all, flushed, pt)
                    flushed = pt

        if loop_reps > 0:
            with tc.For_i(0, loop_reps, 1):
                body()
        else:
            for _ in range(reps):
                body()

    nc.compile()
    return nc


def _get_program(bounds, reps=1):
    key = (bounds, reps)
    if key not in _prog_cache:
        _prog_cache[key] = _build(bounds, reps=reps)
    return _prog_cache[key]


def _make_in_maps(q, k, v, bounds):
    aligned = _aligned(bounds)
    row_tiled = CFG["row_tiled"] and aligned
    if CFG["mm_dtype"] == "bf16":
        import ml_dtypes

        dt = ml_dtypes.bfloat16
    elif CFG["mm_dtype"] == "fp16":
        dt = np.float16
    else:
        dt = np.float32
    in_maps = []
    for h in range(H):
        qh = np.ascontiguousarray(q[0, :, h, :].T.astype(dt))  # [E, L]
        kh = np.ascontiguousarray(k[0, :, h, :].T.astype(dt))  # [E, L]
        if row_tiled:
            qh = np.ascontiguousarray(np.concatenate([qh, qh], axis=0))
            kh = np.ascontiguousarray(np.concatenate([kh, kh], axis=0))
        v1h = np.empty((L, E + 1), dtype=dt)
        v1h[:, :E] = v[0, :, h, :].astype(dt)
        v1h[:, E] = 1.0
        if aligned:
            # swizzle so one SBUF partition holds one row of every k-tile:
            # v1_sw[p, g, e] = v1[g*128 + p, e]
            v1h = np.ascontiguousarray(
                v1h.reshape(L // P, P, E + 1).transpose(1, 0, 2)
            )
        in_maps.append({"qT": qh, "kT": kh, "v1": v1h})
    return in_maps


def kernel(q, k, v, seg_ids):
    from concourse import bass_utils

    q = np.asarray(q, dtype=np.float32)
    k = np.asarray(k, dtype=np.float32)
    v = np.asarray(v, dtype=np.float32)
    seg_ids = np.asarray(seg_ids)

    bounds = _segment_bounds(seg_ids)
    nc = _get_program(bounds)
    in_maps = _make_in_maps(q, k, v, bounds)

    res = bass_utils.run_bass_kernel_spmd(nc, in_maps, core_ids=list(range(NCORES)))

    out = np.empty((1, L, H, E), dtype=np.float32)
    for h in range(H):
        out[0, :, h, :] = res.results[h]["outT"].T
    return out
